# revision 33
# baseline (speedup 1.0000x reference)
"""Multi-head attention block (QKV proj + softmax attention + out proj +
residual + LayerNorm) on 8 Trainium2 NeuronCores.

Sharding: core c -> batch b = c//2, query half = c%2 (512 query rows).
K/V are recomputed per core for its batch (full 1024 keys). No collectives;
each core writes a disjoint [512, 128] slice of the output.

Math notes:
- attn scale sqrt(128) is folded into Wq (and bq) host-side.
- softmax computed without max subtraction: logits are ~±30 so exp fits
  f32/bf16 range comfortably.
- v bias is folded into an adjusted output bias host-side
  (bo_eff = bo + bqkv_v @ Wo), exact because softmax rows sum to 1.
- per-head layout on chip: qT/kT are [head_dim, seq] so scores come out as
  S^T = [sk, sq] which feeds PV directly without transposes; the softmax
  denominator is computed with an all-ones stationary matmul that lands the
  row sums broadcast across all 128 partitions; 1/sum = exp(-ln(sum)) on the
  scalar engine (iterative DVE reciprocal is ~5x slower and stalls PSUM).
- pipeline: projections for head h+1 are emitted while head h's attention
  runs; exp is batched over [128,1024] two-bank PSUM score tiles.
"""

import numpy as np
import ml_dtypes

BS, SEQ, D, H = 4, 1024, 128, 16
INNER = D * H  # 2048
SQ = SEQ // 2  # query rows per core
NCORES = 8
BF16 = ml_dtypes.bfloat16

TRACE = False
LAST_EXEC_NS = None

_CACHED_NC = None


def _split_excess_waits(nc, mybir, max_waits=1):
    # walrus (CoreV3 codegen) rejects instructions carrying too many sem
    # waits (the TileContext end drain accumulates one per live semaphore);
    # move excess waits onto InstNoOp carriers inserted just before.
    for fn in nc.m.functions:
        for blk in fn.blocks:
            idx = 0
            while idx < len(blk.instructions):
                inst = blk.instructions[idx]
                si = getattr(inst, "sync_info", None)
                if si is not None and len(si.on_wait) > max_waits:
                    extra = list(si.on_wait[: len(si.on_wait) - max_waits])
                    si.on_wait = list(si.on_wait[len(si.on_wait) - max_waits :])
                    for w in extra:
                        carrier = mybir.InstNoOp(
                            name=nc.get_next_instruction_name(),
                            sync_info=mybir.SyncInfo(on_wait=[w], on_update=[]),
                            bass_nofuse=True,
                            engine=inst.engine,
                        )
                        nc.register_instruction(carrier)
                        blk.instructions.insert(idx, carrier)
                        idx += 1
                idx += 1


def _build_nc():
    import concourse.bass as bass
    import concourse.tile as tile
    from concourse import mybir

    AF = mybir.ActivationFunctionType
    f32 = mybir.dt.float32
    bf = mybir.dt.bfloat16
    AOP = mybir.AluOpType

    nc = bass.Bass()
    xt = nc.dram_tensor("xt", [D, SEQ], bf, kind="ExternalInput")
    xtq = nc.dram_tensor("xtq", [D, SQ], bf, kind="ExternalInput")
    xres = nc.dram_tensor("xres", [128, SQ], f32, kind="ExternalInput")
    wq = nc.dram_tensor("wq", [D, INNER], bf, kind="ExternalInput")
    wk = nc.dram_tensor("wk", [D, INNER], bf, kind="ExternalInput")
    wv = nc.dram_tensor("wv", [D, INNER], bf, kind="ExternalInput")
    wo = nc.dram_tensor("wo", [D, INNER], bf, kind="ExternalInput")
    bqk = nc.dram_tensor("bqk", [D, 2 * H], f32, kind="ExternalInput")
    vecs = nc.dram_tensor("vecs", [1, 2 * D], f32, kind="ExternalInput")
    y = nc.dram_tensor("y", [SQ, D], f32, kind="ExternalOutput")

    with tile.TileContext(nc) as tc:
        with (
            tc.tile_pool(name="singles", bufs=1) as singles,
            tc.tile_pool(name="work", bufs=2) as work,
            tc.tile_pool(name="psum", bufs=1, space=bass.MemorySpace.PSUM) as psum,
        ):
            # Startup DMAs split across the two TRN2 HW DGE queues (SP +
            # Activation engines) so the initial weight loads run in
            # parallel. Head-0's weight slices get their own small tiles,
            # DMA'd first, so head-0 matmuls start ~1.7us after the queues
            # open instead of waiting for the full 512KB weight transfers.
            wq0_sb = singles.tile([D, 128], bf)
            nc.sync.dma_start(wq0_sb[:], wq[:, 0:128])
            xtq_sb = singles.tile([D, SQ], bf)
            nc.sync.dma_start(xtq_sb[:], xtq[:])
            wq_sb = singles.tile([D, INNER], bf)
            nc.sync.dma_start(wq_sb[:], wq[:])
            wv_sb = singles.tile([D, INNER], bf)
            nc.sync.dma_start(wv_sb[:], wv[:])
            wk0_sb = singles.tile([D, 128], bf)
            nc.scalar.dma_start(wk0_sb[:], wk[:, 0:128])
            xta_sb = singles.tile([D, 512], bf)
            nc.scalar.dma_start(xta_sb[:], xt[:, 0:512])
            bqk_sb = singles.tile([D, 2 * H], f32)
            nc.scalar.dma_start(bqk_sb[:], bqk[:])
            wv0_sb = singles.tile([D, 128], bf)
            nc.scalar.dma_start(wv0_sb[:], wv[:, 0:128])
            xtb_sb = singles.tile([D, 512], bf)
            nc.scalar.dma_start(xtb_sb[:], xt[:, 512:1024])
            wk_sb = singles.tile([D, INNER], bf)
            nc.scalar.dma_start(wk_sb[:], wk[:])
            wo_sb = singles.tile([D, INNER], bf)
            nc.scalar.dma_start(wo_sb[:], wo[:])
            xres_sb = singles.tile([128, SQ], f32)
            nc.scalar.dma_start(xres_sb[:], xres[:])
            vecs_sb = singles.tile([128, 2 * D], f32)
            v_ap = vecs[:]
            vecs_bcast = bass.AP(
                tensor=v_ap.tensor, offset=v_ap.offset, ap=[[0, 128], v_ap.ap[-1]]
            )
            nc.scalar.dma_start(vecs_sb[:], vecs_bcast)
            ones_sb = singles.tile([128, 128], bf)
            nc.vector.memset(ones_sb[:], 1.0)
            eps_sb = singles.tile([128, 1], f32)
            nc.vector.memset(eps_sb[:], 1e-6)
            o_acc = singles.tile([128, SQ], f32)

            def proj_pieces(h):
                # next head's projections split into 5 separately-emittable
                # pieces so their PSUM ("pp") allocations can be spread
                # through the current head's attention instead of bunched at
                # the head boundary (which stalled PE on pp buffer rotation).
                hs = slice(h * 128, (h + 1) * 128)
                wqs = wq0_sb[:, :] if h == 0 else wq_sb[:, hs]
                wks = wk0_sb[:, :] if h == 0 else wk_sb[:, hs]
                wvs = wv0_sb[:, :] if h == 0 else wv_sb[:, hs]
                xth = (xta_sb, xtb_sb)
                qT = work.tile([128, SQ], bf, tag="qT", bufs=2)
                # kT/vv split per half: tile-granularity dep tracking would
                # otherwise make the first scores MM wait for BOTH pk halves
                kTa = work.tile([128, 512], bf, tag="kTa", bufs=2)
                kTb = work.tile([128, 512], bf, tag="kTb", bufs=2)
                vva = work.tile([128, 512], bf, tag="vva", bufs=2)
                vvb = work.tile([128, 512], bf, tag="vvb", bufs=2)
                kT = (kTa, kTb)
                vv = (vva, vvb)

                def pq():
                    qp = psum.tile([128, 512], f32, tag="pp", bufs=2)
                    nc.tensor.matmul(qp[:], wqs, xtq_sb[:], start=True, stop=True)
                    nc.vector.tensor_scalar_add(
                        out=qT[:], in0=qp[:], scalar1=bqk_sb[:, 2 * h : 2 * h + 1]
                    )

                def pk(half):
                    kp = psum.tile([128, 512], f32, tag="pp", bufs=2)
                    nc.tensor.matmul(
                        kp[:], wks, xth[half][:], start=True, stop=True
                    )
                    nc.vector.tensor_scalar_add(
                        out=kT[half][:],
                        in0=kp[:],
                        scalar1=bqk_sb[:, 2 * h + 1 : 2 * h + 2],
                    )

                def pv(half):
                    vp = psum.tile([128, 512], f32, tag="pp", bufs=2)
                    for q in range(4):
                        nc.tensor.matmul(
                            vp[:, q * 128 : (q + 1) * 128],
                            xth[half][:, q * 128 : (q + 1) * 128],
                            wvs,
                            start=True,
                            stop=True,
                        )
                    nc.vector.tensor_copy(out=vv[half][:], in_=vp[:])

                pieces = [pq, lambda: pk(0), lambda: pk(1),
                          lambda: pv(0), lambda: pv(1)]
                return (qT, kT, vv), pieces

            cur, pieces0 = proj_pieces(0)
            # interleave q/k adds with vv copies on the DVE queue: pq, pk0,
            # pv0, pk1, pv1 measured fastest for the first-scores-MM gate
            for p in (pieces0[0], pieces0[1], pieces0[3], pieces0[2], pieces0[4]):
                p()
            for h in range(H):
                qT, kT, vv = cur
                if h + 1 < H:
                    cur, npieces = proj_pieces(h + 1)
                else:
                    npieces = [None] * 5

                # ---- scores + exp (batched over 2 PSUM banks) + PV
                lp = psum.tile([128, 512], f32, tag="lp", bufs=1)
                otp = psum.tile([128, 512], f32, tag="otp", bufs=1)
                exs = []

                def emit_pv(i):
                    # lp first so the Ln on the scalar engine can start as
                    # early as possible after the last exp.
                    for half in range(2):
                        e = exs[i][:, half * 512 : (half + 1) * 512]
                        nc.tensor.matmul(
                            lp[:], ones_sb[:], e,
                            start=(i == 0 and half == 0),
                            stop=(i == 3 and half == 1),
                        )
                    for half in range(2):
                        e = exs[i][:, half * 512 : (half + 1) * 512]
                        c = ((2 * i + half) * 128) % 512
                        nc.tensor.matmul(
                            otp[:],
                            vv[i // 2][:, c : c + 128],
                            e,
                            start=(i == 0 and half == 0),
                            stop=(i == 3 and half == 1),
                        )

                for jj in range(4):
                    if h > 0 and npieces[jj] is not None:
                        npieces[jj]()
                    sp2 = psum.tile([128, 1024], f32, tag="sp2", bufs=2)
                    kb = ((2 * jj) * 128) % 512
                    nc.tensor.matmul(
                        sp2[:, 0:512],
                        kT[jj // 2][:, kb : kb + 128],
                        qT[:],
                        start=True,
                        stop=True,
                    )
                    nc.tensor.matmul(
                        sp2[:, 512:1024],
                        kT[jj // 2][:, kb + 128 : kb + 256],
                        qT[:],
                        start=True,
                        stop=True,
                    )
                    if jj > 0:
                        emit_pv(jj - 1)
                    ex = work.tile([128, 1024], bf, tag="ex", bufs=3)
                    nc.scalar.activation(ex[:], sp2[:], AF.Exp)
                    exs.append(ex)
                if h > 0 and npieces[4] is not None:
                    npieces[4]()
                emit_pv(3)
                if h == 0:
                    # head-1 pieces wait on the full weight DMAs; keep them
                    # behind head-0's attention in the in-order PE queue.
                    for p in npieces:
                        p()

                lg = work.tile([128, 512], f32, tag="lg", bufs=2)
                nc.scalar.activation(lg[:], lp[:], AF.Ln)
                linv = work.tile([128, 512], f32, tag="linv", bufs=2)
                nc.scalar.activation(linv[:], lg[:], AF.Exp, scale=-1.0)
                on = work.tile([128, SQ], bf, tag="on", bufs=2)
                nc.vector.tensor_mul(out=on[:], in0=otp[:], in1=linv[:])

                # ---- this head's output-projection contribution, folded into
                # the loop: oc[sq_block, D] quarters, then o_acc += oc on DVE
                oc = psum.tile([128, 512], f32, tag="otp", bufs=1)
                for t in range(4):
                    nc.tensor.matmul(
                        oc[:, t * 128 : (t + 1) * 128],
                        on[:, t * 128 : (t + 1) * 128],
                        wo_sb[:, h * 128 : (h + 1) * 128],
                        start=True,
                        stop=True,
                    )
                if h == 0:
                    # fold the residual in at init so the epilogue skips it
                    nc.vector.tensor_add(out=o_acc[:], in0=oc[:], in1=xres_sb[:])
                else:
                    nc.vector.tensor_add(out=o_acc[:], in0=o_acc[:], in1=oc[:])

            # ---- epilogue: LayerNorm (residual+bias already in o_acc), store.
            # rstd = exp(-0.5*ln(var+eps)) keeps the work on the scalar
            # engine (Ln/Exp share the already-loaded ACT table; Sqrt would
            # force a 1.3us table reload) and skips the DVE reciprocal.
            # stage-major: all stats first, then all rstd ACT roundtrips,
            # then all normalize+store -- so the in-order DVE queue never
            # stalls waiting on the ACT Ln/Exp roundtrip per block.
            mvs, rstds = [], []
            for t in range(4):
                ts = slice(t * 128, (t + 1) * 128)
                stats = work.tile([128, 6], f32)
                nc.vector.bn_stats(out=stats[:], in_=o_acc[:, ts])
                mv = work.tile([128, 2], f32, tag=f"mv{t}", bufs=1)
                nc.vector.bn_aggr(out=mv[:], in_=stats[:])
                mvs.append(mv)
            for t in range(4):
                lnv = work.tile([128, 1], f32)
                nc.scalar.activation(
                    out=lnv[:], in_=mvs[t][:, 1:2], func=AF.Ln, bias=eps_sb[:]
                )
                rstd = work.tile([128, 1], f32, tag=f"rstd{t}", bufs=1)
                nc.scalar.activation(
                    out=rstd[:], in_=lnv[:], func=AF.Exp, scale=-0.5
                )
                rstds.append(rstd)
            for t in range(4):
                ts = slice(t * 128, (t + 1) * 128)
                norm = work.tile([128, 128], f32, tag=f"norm{t}", bufs=1)
                nc.vector.tensor_scalar(
                    out=norm[:],
                    in0=o_acc[:, ts],
                    scalar1=mvs[t][:, 0:1],
                    scalar2=rstds[t][:],
                    op0=AOP.subtract,
                    op1=AOP.mult,
                )
                nc.vector.tensor_mul(out=norm[:], in0=norm[:], in1=vecs_sb[:, 0:D])
                nc.vector.tensor_add(
                    out=norm[:], in0=norm[:], in1=vecs_sb[:, D : 2 * D]
                )
                eng = nc.sync if t % 2 == 0 else nc.scalar
                eng.dma_start(y[ts, :], norm[:])

    _split_excess_waits(nc, mybir)
    return nc


def kernel(X, Wqkv, bqkv, Wo, bo, ln_scale, ln_bias):
    global _CACHED_NC, LAST_EXEC_NS
    from concourse.bass_utils import run_bass_kernel_spmd

    if _CACHED_NC is None:
        _CACHED_NC = _build_nc()
    nc = _CACHED_NC

    X = np.asarray(X, np.float32)
    Wqkv = np.asarray(Wqkv, np.float32)
    bqkv = np.asarray(bqkv, np.float32)
    Wo = np.asarray(Wo, np.float32)
    bo = np.asarray(bo, np.float32)
    ln_scale = np.asarray(ln_scale, np.float32)
    ln_bias = np.asarray(ln_bias, np.float32)

    sqd = np.float32(np.sqrt(float(D)))
    wq_h = (Wqkv[:, :INNER] * sqd).astype(BF16)
    wk_h = Wqkv[:, INNER : 2 * INNER].astype(BF16)
    wv_h = Wqkv[:, 2 * INNER :].astype(BF16)
    # head-major re-layout: wo_re[p, h*128+n] = Wo[h*128+p, n]
    wo_re = np.ascontiguousarray(
        Wo.reshape(H, 128, D).transpose(1, 0, 2).reshape(128, INNER)
    ).astype(BF16)
    bo_eff = (bo + bqkv[2 * INNER :] @ Wo).astype(np.float32)
    bq = Wqkv.dtype.type(0) + bqkv[:INNER].reshape(H, 128) * sqd
    bk = bqkv[INNER : 2 * INNER].reshape(H, 128)
    bqk_h = np.zeros((128, 2 * H), np.float32)
    bqk_h[:, 0::2] = bq.T
    bqk_h[:, 1::2] = bk.T
    vecs_h = np.concatenate([ln_scale, ln_bias])[None, :].astype(np.float32)
    bo_tiled = np.tile(bo_eff, 4)[None, :]  # folded into xres per-core below

    in_maps = []
    for c in range(NCORES):
        b, half = c // 2, c % 2
        q0 = half * SQ
        XT = np.ascontiguousarray(X[b].T)  # [128, 1024] f32
        xt_h = XT.astype(BF16)
        xtq_h = np.ascontiguousarray(XT[:, q0 : q0 + SQ]).astype(BF16)
        # xres[p, t*128+cc] = X[b][q0 + t*128 + p, cc]
        xres_h = np.ascontiguousarray(
            X[b][q0 : q0 + SQ].reshape(4, 128, 128).transpose(1, 0, 2).reshape(128, SQ)
        ) + bo_tiled
        in_maps.append(
            dict(
                xt=xt_h, xtq=xtq_h, xres=xres_h,
                wq=wq_h, wk=wk_h, wv=wv_h, wo=wo_re,
                bqk=bqk_h, vecs=vecs_h,
            )
        )

    res = run_bass_kernel_spmd(nc, in_maps, core_ids=list(range(NCORES)), trace=TRACE)
    LAST_EXEC_NS = res.exec_time_ns

    out = np.empty((BS, SEQ, D), np.float32)
    for c in range(NCORES):
        b, half = c // 2, c % 2
        out[b, half * SQ : (half + 1) * SQ, :] = res.results[c]["y"]
    return out
